# revision 16
# baseline (speedup 1.0000x reference)
"""BoundaryLoss kernel for 8 Trainium2 NeuronCores (v3).

loss = sum_c mean_{b,h,w}((|sobel(labels_c)| - |sobel(probs_c)|)^2)

Data-parallel: core k processes batches [2k, 2k+1] x classes 1..4
(8 image pairs of 512x512). Per-core partial sums are combined on host.

v3: the per-dma_start fixed cost (~2-3us, serialized per HWDGE ring)
dominated earlier versions (baseline: 84 DMAs ~= 170us). The host now
pre-packs SBUF-layout pair tiles (row-band blocks with halo rows and
zero pads), so the device issues only 9 big contiguous DMAs per loop
iteration, split across the two HWDGE rings (nc.sync / nc.scalar).

Per-iteration compute (measured-cost-balanced):
  - TensorE: 10 fp16 band-matrix matmuls -> PSUM [gx_l|gy_l|gx_p|gy_p].
  - ACT: one batched Square over PSUM cols [0:ACT_SQ_COLS] (+accum).
  - DVE: copy PSUM cols [ACT_SQ_COLS:2048] -> fp16, STT square (+accum).
  - DVE (lagged 1 iter): m = gx^2+gy^2 (one TT), q = m_l*m_p into a
    4-iter q batch.
  - ACT (lagged, per 4 iters): Sqrt over the q batch (+accum).
  loss*B*H*W = sum(acc_a) + sum(acc_c) - 2*sum(acc_b), combined on host.
"""

import sys

import numpy as np

if "/opt/trn_rl_repo" not in sys.path:
    sys.path.insert(0, "/opt/trn_rl_repo")

from contextlib import ExitStack

import concourse.bass as bass
import concourse.mybir as mybir
import concourse.tile as tile

H = W = 512
N_IMG = 8          # image pairs per core
BAND = 126         # output rows per full band
N_BANDS = 4        # full 126-row bands; bottom 8 rows via 2 packed iters
N_ITERS = N_IMG * N_BANDS + 2
PADW = W + 2       # padded columns per block
BLK = PADW         # block stride inside a pair tile
PAIRW = 2 * 4 * BLK  # columns per pair tile (2 sides x 4 blocks)
SMOOTH = 1e-6
# columns of the 2048-wide PSUM handled by ACT Square (rest via DVE).
# Must be PSUM-bank aligned (multiple of 512): ScalarE and VectorE can only
# access PSUM concurrently when they touch different banks.
ACT_SQ_COLS = 2048
QB = 16            # iters per sqrt batch
SQRT_LAG = 2       # sqrt of batch b is emitted during iter QB*b+QB+SQRT_LAG

F32 = mybir.dt.float32
F16 = mybir.dt.float16


def _stationaries():
    """lhsT weight matrices [p, c]: moving partition p -> out partition c."""
    bv = np.zeros((128, 128), np.float32)   # vertical smooth [1,2,1]
    bdf = np.zeros((128, 128), np.float32)  # vertical diff [1,0,-1]
    for c in range(126):
        bv[c, c] = 1.0
        bv[c + 1, c] = 2.0
        bv[c + 2, c] = 1.0
        bdf[c, c] = 1.0
        bdf[c + 2, c] = -1.0
    # Packed bottom-band versions: 4 images per iteration; image k's rows
    # 503..511 live at input partitions 16k..16k+8 (16k+9 is the zeroed
    # row-512 halo), outputs 504..511 at partitions 8k..8k+7.
    bvm = np.zeros((128, 128), np.float32)
    bdfm = np.zeros((128, 128), np.float32)
    for k in range(4):
        for i in range(8):
            bvm[16 * k + i, 8 * k + i] = 1.0
            bvm[16 * k + i + 1, 8 * k + i] = 2.0
            bvm[16 * k + i + 2, 8 * k + i] = 1.0
            bdfm[16 * k + i, 8 * k + i] = 1.0
            bdfm[16 * k + i + 2, 8 * k + i] = -1.0
    return np.concatenate(
        [bv, -bv, bdf, 2.0 * bdf, bvm, -bvm, bdfm, 2.0 * bdfm],
        axis=1).astype(np.float16)


def pack_host_inputs(l4, p4):
    """Build the SBUF-layout host tensors.

    l4, p4: float16 [8, 512, 512] (labels / probs images for this core).
    Returns xs [8, 128, PAIRW] and xmh [128, 4 * BLK].
    """
    xs = np.zeros((N_IMG, 128, PAIRW), np.float16)
    for i in range(N_IMG):
        for s, img in ((0, l4[i]), (1, p4[i])):
            c0 = (4 * s) * BLK
            xs[i, 1:128, c0 + 1:c0 + 1 + W] = img[0:127]
            for b in range(1, 4):
                cb = (4 * s + b) * BLK
                r0 = 126 * b - 1
                xs[i, :, cb + 1:cb + 1 + W] = img[r0:r0 + 128]
    xmh = np.zeros((128, 4 * BLK), np.float16)
    for q in range(2):
        for s, arr in ((0, l4), (1, p4)):
            j = 2 * q + s
            for k in range(4):
                xmh[16 * k:16 * k + 9, j * BLK + 1:j * BLK + 1 + W] = \
                    arr[4 * q + k, 503:512]
    return xs, xmh


def _split_waits_json(bir: bytes, maxw: int = 1) -> bytes:
    """Walrus in this container rejects instructions with >1 semaphore wait
    ("Too many sync wait commands"). Split extra waits onto NoOp carriers
    inserted just before the instruction on the same engine — semantics are
    identical (same waits, same order, before the instruction executes)."""
    import orjson

    d = orjson.loads(bir)
    ctr = 0
    for fn in d["functions"]:
        for b in fn["blocks"]:
            new = []
            for ins in b["instructions"]:
                si = ins.get("sync_info")
                if si:
                    waits = si.get("on_wait") or []
                    if len(waits) > maxw:
                        keep = waits[-maxw:] if maxw else []
                        for w in waits[: len(waits) - maxw]:
                            ctr += 1
                            new.append({
                                "debug": ins.get("debug", 0),
                                "engine": ins["engine"],
                                "ins": [],
                                "outs": [],
                                "name": f"{ins['name']}-wsplit{ctr}",
                                "opcode": "NoOp",
                                "sync_info": {"on_wait": [w], "on_update": []},
                            })
                        si["on_wait"] = keep
                new.append(ins)
            b["instructions"] = new
    return orjson.dumps(d)


def _patch_serialization(nc):
    fixed = _split_waits_json(nc.to_json_bytes())
    nc.to_json_bytes = lambda: fixed
    return nc


def build_kernel(loop: int = 1, variant: str = "full"):
    nc = bass.Bass()
    xs = nc.dram_tensor("xs", [N_IMG, 128, PAIRW], F16, kind="ExternalInput")
    xmh = nc.dram_tensor("xmh", [128, 4 * BLK], F16, kind="ExternalInput")
    consts = nc.dram_tensor("consts", [128, 1024], F16, kind="ExternalInput")
    out = nc.dram_tensor("out", [128, 3], F32, kind="ExternalOutput")

    with ExitStack() as ctx:
        tc = ctx.enter_context(tile.TileContext(nc))
        cpool = ctx.enter_context(tc.tile_pool(name="consts", bufs=1))
        xpool = ctx.enter_context(tc.tile_pool(name="x", bufs=1))
        psum_pool = ctx.enter_context(tc.tile_pool(name="g", bufs=2, space="PSUM"))
        sq_pool = ctx.enter_context(tc.tile_pool(name="sq", bufs=4))
        c16_pool = ctx.enter_context(tc.tile_pool(name="c16", bufs=4))
        m_pool = ctx.enter_context(tc.tile_pool(name="m", bufs=4))
        q_pool = ctx.enter_context(tc.tile_pool(name="q", bufs=1))
        acc_pool = ctx.enter_context(tc.tile_pool(name="acc", bufs=1))

        wmat = cpool.tile([128, 1024], F16, tag="wmat")
        nc.sync.dma_start(out=wmat[:, :], in_=consts[:, :])
        (BV, BVN, BDF, BDF2, BVM, BVNM, BDFM, BDF2M) = (
            wmat[:, 128 * i:128 * i + 128] for i in range(8))

        acc_a = acc_pool.tile([128, N_ITERS], F32, tag="acc_a")
        acc_c = acc_pool.tile([128, N_ITERS], F32, tag="acc_c")
        acc_b = acc_pool.tile([128, N_ITERS // QB + 1], F32, tag="acc_b")
        nc.vector.memset(acc_a[:, :], 0.0)
        nc.vector.memset(acc_b[:, :], 0.0)
        nc.vector.memset(acc_c[:, :], 0.0)
        out_s = acc_pool.tile([128, 3], F32, tag="out_s")

        # Per-(pair, side) tiles + one packed-bottom tile. Layout comes
        # pre-built from the host (halos, zero pads included). Separate
        # tiles keep DMA->matmul dependencies fine-grained so compute on
        # pair 0 overlaps the remaining input DMAs.
        st = [xpool.tile([128, 4 * BLK], F16, name=f"st{j}", tag=f"st{j}")
              for j in range(2 * N_IMG)]
        xmt = xpool.tile([128, 4 * BLK], F16, tag="xmt")

        def blk(i, s, b):
            """Block AP [128, BLK] of pair i, side s, block b."""
            return st[2 * i + s][:, b * BLK:(b + 1) * BLK]

        def emit_dmas():
            # All input DMAs on the SP (sync) HWDGE ring (nc.scalar would
            # stall the ACT Square stream; SWDGE/gpsimd doesn't compile in
            # this container). Pair 0 is split per band block so the first
            # matmuls can start after ~2.5us instead of ~9us.
            # pair 0: block 0 of each side first (lead ~2.6us), then rest
            for s in range(2):
                nc.sync.dma_start(
                    out=st[s][:, 0:BLK],
                    in_=xs[0, :, (4 * s) * BLK:(4 * s + 1) * BLK])
            for s in range(2):
                nc.sync.dma_start(
                    out=st[s][:, BLK:4 * BLK],
                    in_=xs[0, :, (4 * s + 1) * BLK:(4 * s + 4) * BLK])
            for i in range(1, N_IMG):
                nc.sync.dma_start(
                    out=st[2 * i][:, :], in_=xs[i, :, 0:4 * BLK])
                nc.sync.dma_start(
                    out=st[2 * i + 1][:, :], in_=xs[i, :, 4 * BLK:PAIRW])
            nc.sync.dma_start(out=xmt[:, :], in_=xmh[:, :])

        def emit_mms(g, xlr, xpr, stat, pv, kp):
            # Stationary-major order: 4 weight loads per iteration.
            sv, svn, sdf, sdf2 = stat
            xx = ((xlr, 0), (xpr, 1024))
            for x_, c in xx:
                nc.tensor.matmul(g[0:pv, c:c + 512], sv[0:kp, 0:pv],
                                 x_[0:kp, 0:W], start=True, stop=False)
            for x_, c in xx:
                nc.tensor.matmul(g[0:pv, c:c + 512], svn[0:kp, 0:pv],
                                 x_[0:kp, 2:2 + W], start=False, stop=True)
            for x_, c in xx:
                nc.tensor.matmul(g[0:pv, c + 512:c + 1024], sdf[0:kp, 0:pv],
                                 x_[0:kp, 0:W], start=True, stop=False)
                nc.tensor.matmul(g[0:pv, c + 512:c + 1024], sdf[0:kp, 0:pv],
                                 x_[0:kp, 2:2 + W], start=False, stop=False)
            for x_, c in xx:
                nc.tensor.matmul(g[0:pv, c + 512:c + 1024], sdf2[0:kp, 0:pv],
                                 x_[0:kp, 1:1 + W], start=False, stop=True)

        loop_ctx = tc.For_i(0, loop, 1) if loop > 1 else None
        if loop_ctx is not None:
            loop_ctx.__enter__()

        if variant != "mm":
            emit_dmas()

        # Deferred per-iteration stages, emitted with a lag so engines
        # never wait on each other within an iteration.
        pending = []          # (sq, pv, it) waiting for madd/qmul
        qtiles = {}           # batch index -> q tile
        qfill = {}            # batch index -> number of filled slots

        def do_madd_qmul(sq, pv, it):
            m = m_pool.tile([128, 1024], F16)
            sqv = sq.rearrange("p (a b c) -> p a b c", a=2, b=2, c=512)
            mv = m.rearrange("p (a c) -> p a c", a=2, c=512)
            nc.vector.tensor_add(mv[0:pv, :, :], sqv[0:pv, :, 0, :],
                                 sqv[0:pv, :, 1, :])
            b, slot = divmod(it, QB)
            if slot == 0:
                qtiles[b] = q_pool.tile([128, QB * 512], F16, name=f"q{b}")
            q = qtiles[b]
            qfill[b] = slot + 1
            nc.vector.tensor_mul(q[0:pv, slot * 512:slot * 512 + 512],
                                 m[0:pv, 0:512], m[0:pv, 512:1024])
            if pv < 126:
                # zero unused partitions so the batched sqrt+accum over
                # [0:126] rows stays clean (packed-bottom iters, pv=32);
                # memset APs must start 32-aligned and span <= 32 partitions
                for p0, p1 in ((32, 64), (64, 96), (96, 126)):
                    nc.vector.memset(q[p0:p1, slot * 512:slot * 512 + 512], 0.0)

        def do_sqrt(b):
            q = qtiles.pop(b)
            w = qfill.pop(b) * 512
            nc.scalar.activation(q[0:126, 0:w], q[0:126, 0:w],
                                 mybir.ActivationFunctionType.Sqrt,
                                 accum_out=acc_b[0:126, b:b + 1])

        it = 0
        for phase in range(N_IMG + 2):
            if phase < N_IMG:
                img = phase
                bands = range(N_BANDS)
            else:
                bands = (-1,)
            for t in bands:
                if t >= 0:
                    xlr = blk(img, 0, t)
                    xpr = blk(img, 1, t)
                    stat, pv, kp = (BV, BVN, BDF, BDF2), BAND, 128
                else:
                    q2 = phase - N_IMG
                    xlr = xmt[:, (2 * q2) * BLK:(2 * q2) * BLK + BLK]
                    xpr = xmt[:, (2 * q2 + 1) * BLK:(2 * q2 + 1) * BLK + BLK]
                    stat, pv, kp = (BVM, BVNM, BDFM, BDF2M), 32, 58

                if variant == "dma":
                    it += 1
                    continue
                # PSUM layout: [gx_l | gy_l | gx_p | gy_p], 512 f32 each.
                g = psum_pool.tile([128, 2048], F32)
                emit_mms(g, xlr, xpr, stat, pv, kp)

                if variant in ("dma_mm", "mm"):
                    # cheap PSUM release: copy 64 cols per bank-pair
                    sqm = sq_pool.tile([128, 2048], F16)
                    nc.vector.tensor_copy(
                        sqm[0:126, 0:64], g[0:126, 0:64])
                    nc.vector.tensor_copy(
                        sqm[0:126, 1024:1088], g[0:126, 1024:1088])
                    nc.vector.tensor_reduce(
                        acc_a[0:126, it:it + 1], sqm[0:126, 0:64],
                        axis=mybir.AxisListType.X, op=mybir.AluOpType.add)
                    it += 1
                    continue
                if variant == "evict_actonly":
                    sqa = sq_pool.tile([128, 2048], F16)
                    nc.scalar.activation(sqa[0:pv, :], g[0:pv, :],
                                         mybir.ActivationFunctionType.Square,
                                         accum_out=acc_a[0:pv, it:it + 1])
                    it += 1
                    continue
                if variant == "evict_dveonly":
                    sqd = sq_pool.tile([128, 2048], F16)
                    nc.vector.tensor_copy(sqd[0:pv, :], g[0:pv, :])
                    nc.vector.scalar_tensor_tensor(
                        out=sqd[0:pv, 0:1024], in0=sqd[0:pv, 0:1024],
                        scalar=1.0, in1=sqd[0:pv, 0:1024],
                        op0=mybir.AluOpType.mult, op1=mybir.AluOpType.mult,
                        accum_out=acc_c[0:pv, it:it + 1])
                    it += 1
                    continue
                # Eviction: ACT squares the whole PSUM tile in one batched
                # op (+accum of sum(gx^2+gy^2)). The DVE never touches PSUM:
                # its PSUM reads measurably serialize against PE matmuls in
                # this toolchain, while ACT-PSUM reads overlap them cleanly.
                sq = sq_pool.tile([128, 2048], F16)
                nc.scalar.activation(sq[0:pv, :], g[0:pv, :],
                                     mybir.ActivationFunctionType.Square,
                                     accum_out=acc_a[0:pv, it:it + 1])

                if variant == "evict":
                    it += 1
                    continue
                # Lagged DVE madd/qmul (previous iteration's sq).
                pending.append((sq, pv, it))
                if len(pending) > 1:
                    do_madd_qmul(*pending.pop(0))
                # Lagged ACT sqrt over completed q batches.
                bq = (it - QB - SQRT_LAG) // QB
                if it % QB == SQRT_LAG and bq >= 0 and bq in qtiles:
                    do_sqrt(bq)
                it += 1

        while pending:
            do_madd_qmul(*pending.pop(0))
        for b in sorted(qtiles):
            do_sqrt(b)
        if variant == "dma":
            # consume the tiles so the DMAs stay live
            for j in range(2 * N_IMG):
                nc.vector.tensor_reduce(
                    acc_a[:, j // 2:j // 2 + 1], st[j][:, 0:256],
                    axis=mybir.AxisListType.X, op=mybir.AluOpType.add)
            nc.vector.tensor_reduce(acc_a[:, 8:9], xmt[:, :],
                                    axis=mybir.AxisListType.X,
                                    op=mybir.AluOpType.add)

        if loop_ctx is not None:
            loop_ctx.__exit__(None, None, None)
        nc.vector.tensor_reduce(out_s[:, 0:1], acc_a[:, :],
                                axis=mybir.AxisListType.X, op=mybir.AluOpType.add)
        nc.vector.tensor_reduce(out_s[:, 1:2], acc_b[:, :],
                                axis=mybir.AxisListType.X, op=mybir.AluOpType.add)
        nc.vector.tensor_reduce(out_s[:, 2:3], acc_c[:, :],
                                axis=mybir.AxisListType.X, op=mybir.AluOpType.add)
        nc.sync.dma_start(out=out[:, :], in_=out_s[:, :])
    return _patch_serialization(nc)


_NC = None


def kernel(probs, labels):
    global _NC
    from concourse.bass_utils import run_bass_kernel_spmd

    if _NC is None:
        _NC = build_kernel()

    p = np.asarray(probs)[:, 1:5].astype(np.float16)
    l = np.asarray(labels)[:, 1:5].astype(np.float16)
    wmat = _stationaries()

    in_maps = []
    for k in range(8):
        l4 = l[2 * k:2 * k + 2].reshape(N_IMG, H, W)
        p4 = p[2 * k:2 * k + 2].reshape(N_IMG, H, W)
        xs_h, xm_h = pack_host_inputs(l4, p4)
        in_maps.append({"xs": xs_h, "xmh": xm_h, "consts": wmat})
    res = run_bass_kernel_spmd(_NC, in_maps, list(range(8)))
    total = 0.0
    for r in res.results:
        o = r["out"].astype(np.float64)
        total += o[:, 0].sum() + o[:, 2].sum() - 2.0 * o[:, 1].sum()
    return np.float32(total / (16 * H * W))


# revision 17
# speedup vs baseline: 1.0562x; 1.0562x over previous
"""BoundaryLoss kernel for 8 Trainium2 NeuronCores (v3).

loss = sum_c mean_{b,h,w}((|sobel(labels_c)| - |sobel(probs_c)|)^2)

Data-parallel: core k processes batches [2k, 2k+1] x classes 1..4
(8 image pairs of 512x512). Per-core partial sums are combined on host.

v3: the per-dma_start fixed cost (~2-3us, serialized per HWDGE ring)
dominated earlier versions (baseline: 84 DMAs ~= 170us). The host now
pre-packs SBUF-layout pair tiles (row-band blocks with halo rows and
zero pads), so the device issues only 9 big contiguous DMAs per loop
iteration, split across the two HWDGE rings (nc.sync / nc.scalar).

Per-iteration compute (measured-cost-balanced):
  - TensorE: 10 fp16 band-matrix matmuls -> PSUM [gx_l|gy_l|gx_p|gy_p].
  - ACT: one batched Square over PSUM cols [0:ACT_SQ_COLS] (+accum).
  - DVE: copy PSUM cols [ACT_SQ_COLS:2048] -> fp16, STT square (+accum).
  - DVE (lagged 1 iter): m = gx^2+gy^2 (one TT), q = m_l*m_p into a
    4-iter q batch.
  - ACT (lagged, per 4 iters): Sqrt over the q batch (+accum).
  loss*B*H*W = sum(acc_a) + sum(acc_c) - 2*sum(acc_b), combined on host.
"""

import sys

import numpy as np

if "/opt/trn_rl_repo" not in sys.path:
    sys.path.insert(0, "/opt/trn_rl_repo")

from contextlib import ExitStack

import concourse.bass as bass
import concourse.mybir as mybir
import concourse.tile as tile

H = W = 512
N_IMG = 8          # image pairs per core
BAND = 126         # output rows per full band
N_BANDS = 4        # full 126-row bands; bottom 8 rows via 2 packed iters
N_ITERS = N_IMG * N_BANDS + 2
PADW = W + 2       # padded columns per block
BLK = PADW         # block stride inside a pair tile
PAIRW = 2 * 4 * BLK  # columns per pair tile (2 sides x 4 blocks)
SMOOTH = 1e-6
# columns of the 2048-wide PSUM handled by ACT Square (rest via DVE).
# Must be PSUM-bank aligned (multiple of 512): ScalarE and VectorE can only
# access PSUM concurrently when they touch different banks.
ACT_SQ_COLS = 2048
QB = 8             # iters per sqrt batch
SQRT_LAG = 0       # sqrt of batch b is emitted during iter QB*b+QB+SQRT_LAG

F32 = mybir.dt.float32
F16 = mybir.dt.float16


def _stationaries():
    """lhsT weight matrices [p, c]: moving partition p -> out partition c."""
    bv = np.zeros((128, 128), np.float32)   # vertical smooth [1,2,1]
    bdf = np.zeros((128, 128), np.float32)  # vertical diff [1,0,-1]
    for c in range(126):
        bv[c, c] = 1.0
        bv[c + 1, c] = 2.0
        bv[c + 2, c] = 1.0
        bdf[c, c] = 1.0
        bdf[c + 2, c] = -1.0
    # Packed bottom-band versions: 4 images per iteration; image k's rows
    # 503..511 live at input partitions 16k..16k+8 (16k+9 is the zeroed
    # row-512 halo), outputs 504..511 at partitions 8k..8k+7.
    bvm = np.zeros((128, 128), np.float32)
    bdfm = np.zeros((128, 128), np.float32)
    for k in range(4):
        for i in range(8):
            bvm[16 * k + i, 8 * k + i] = 1.0
            bvm[16 * k + i + 1, 8 * k + i] = 2.0
            bvm[16 * k + i + 2, 8 * k + i] = 1.0
            bdfm[16 * k + i, 8 * k + i] = 1.0
            bdfm[16 * k + i + 2, 8 * k + i] = -1.0
    return np.concatenate(
        [bv, -bv, bdf, 2.0 * bdf, bvm, -bvm, bdfm, 2.0 * bdfm],
        axis=1).astype(np.float16)


def pack_host_inputs(l4, p4):
    """Build the SBUF-layout host tensors.

    l4, p4: float16 [8, 512, 512] (labels / probs images for this core).
    Returns xs [8, 128, PAIRW] and xmh [128, 4 * BLK].
    """
    xs = np.zeros((N_IMG, 128, PAIRW), np.float16)
    for i in range(N_IMG):
        for s, img in ((0, l4[i]), (1, p4[i])):
            c0 = (4 * s) * BLK
            xs[i, 1:128, c0 + 1:c0 + 1 + W] = img[0:127]
            for b in range(1, 4):
                cb = (4 * s + b) * BLK
                r0 = 126 * b - 1
                xs[i, :, cb + 1:cb + 1 + W] = img[r0:r0 + 128]
    xmh = np.zeros((128, 4 * BLK), np.float16)
    for q in range(2):
        for s, arr in ((0, l4), (1, p4)):
            j = 2 * q + s
            for k in range(4):
                xmh[16 * k:16 * k + 9, j * BLK + 1:j * BLK + 1 + W] = \
                    arr[4 * q + k, 503:512]
    return xs, xmh


def _split_waits_json(bir: bytes, maxw: int = 1) -> bytes:
    """Walrus in this container rejects instructions with >1 semaphore wait
    ("Too many sync wait commands"). Split extra waits onto NoOp carriers
    inserted just before the instruction on the same engine — semantics are
    identical (same waits, same order, before the instruction executes)."""
    import orjson

    d = orjson.loads(bir)
    ctr = 0
    for fn in d["functions"]:
        for b in fn["blocks"]:
            new = []
            for ins in b["instructions"]:
                si = ins.get("sync_info")
                if si:
                    waits = si.get("on_wait") or []
                    if len(waits) > maxw:
                        keep = waits[-maxw:] if maxw else []
                        for w in waits[: len(waits) - maxw]:
                            ctr += 1
                            new.append({
                                "debug": ins.get("debug", 0),
                                "engine": ins["engine"],
                                "ins": [],
                                "outs": [],
                                "name": f"{ins['name']}-wsplit{ctr}",
                                "opcode": "NoOp",
                                "sync_info": {"on_wait": [w], "on_update": []},
                            })
                        si["on_wait"] = keep
                new.append(ins)
            b["instructions"] = new
    return orjson.dumps(d)


def _patch_serialization(nc):
    fixed = _split_waits_json(nc.to_json_bytes())
    nc.to_json_bytes = lambda: fixed
    return nc


def build_kernel(loop: int = 1, variant: str = "full"):
    nc = bass.Bass()
    xs = nc.dram_tensor("xs", [N_IMG, 128, PAIRW], F16, kind="ExternalInput")
    xmh = nc.dram_tensor("xmh", [128, 4 * BLK], F16, kind="ExternalInput")
    consts = nc.dram_tensor("consts", [128, 1024], F16, kind="ExternalInput")
    out = nc.dram_tensor("out", [128, 3], F32, kind="ExternalOutput")

    with ExitStack() as ctx:
        tc = ctx.enter_context(tile.TileContext(nc))
        cpool = ctx.enter_context(tc.tile_pool(name="consts", bufs=1))
        xpool = ctx.enter_context(tc.tile_pool(name="x", bufs=1))
        psum_pool = ctx.enter_context(tc.tile_pool(name="g", bufs=2, space="PSUM"))
        sq_pool = ctx.enter_context(tc.tile_pool(name="sq", bufs=4))
        c16_pool = ctx.enter_context(tc.tile_pool(name="c16", bufs=4))
        m_pool = ctx.enter_context(tc.tile_pool(name="m", bufs=4))
        q_pool = ctx.enter_context(tc.tile_pool(name="q", bufs=1))
        acc_pool = ctx.enter_context(tc.tile_pool(name="acc", bufs=1))

        wmat = cpool.tile([128, 1024], F16, tag="wmat")
        nc.sync.dma_start(out=wmat[:, :], in_=consts[:, :])
        (BV, BVN, BDF, BDF2, BVM, BVNM, BDFM, BDF2M) = (
            wmat[:, 128 * i:128 * i + 128] for i in range(8))

        acc_a = acc_pool.tile([128, N_ITERS], F32, tag="acc_a")
        acc_c = acc_pool.tile([128, N_ITERS], F32, tag="acc_c")
        acc_b = acc_pool.tile([128, N_ITERS // QB + 1], F32, tag="acc_b")
        nc.vector.memset(acc_a[:, :], 0.0)
        nc.vector.memset(acc_b[:, :], 0.0)
        nc.vector.memset(acc_c[:, :], 0.0)
        out_s = acc_pool.tile([128, 3], F32, tag="out_s")

        # Per-(pair, side) tiles + one packed-bottom tile. Layout comes
        # pre-built from the host (halos, zero pads included). Separate
        # tiles keep DMA->matmul dependencies fine-grained so compute on
        # pair 0 overlaps the remaining input DMAs.
        st = [xpool.tile([128, 4 * BLK], F16, name=f"st{j}", tag=f"st{j}")
              for j in range(2 * N_IMG)]
        xmt = xpool.tile([128, 4 * BLK], F16, tag="xmt")

        def blk(i, s, b):
            """Block AP [128, BLK] of pair i, side s, block b."""
            return st[2 * i + s][:, b * BLK:(b + 1) * BLK]

        def emit_dmas():
            # All input DMAs on the SP (sync) HWDGE ring (nc.scalar would
            # stall the ACT Square stream; SWDGE/gpsimd doesn't compile in
            # this container). Pair 0 is split per band block so the first
            # matmuls can start after ~2.5us instead of ~9us.
            # pair 0: block 0 of each side first (lead ~2.6us), then rest
            for s in range(2):
                nc.sync.dma_start(
                    out=st[s][:, 0:BLK],
                    in_=xs[0, :, (4 * s) * BLK:(4 * s + 1) * BLK])
            for s in range(2):
                nc.sync.dma_start(
                    out=st[s][:, BLK:4 * BLK],
                    in_=xs[0, :, (4 * s + 1) * BLK:(4 * s + 4) * BLK])
            for i in range(1, N_IMG):
                nc.sync.dma_start(
                    out=st[2 * i][:, :], in_=xs[i, :, 0:4 * BLK])
                nc.sync.dma_start(
                    out=st[2 * i + 1][:, :], in_=xs[i, :, 4 * BLK:PAIRW])
            nc.sync.dma_start(out=xmt[:, :], in_=xmh[:, :])

        def emit_mms(g, xlr, xpr, stat, pv, kp):
            # Stationary-major order: 4 weight loads per iteration.
            sv, svn, sdf, sdf2 = stat
            xx = ((xlr, 0), (xpr, 1024))
            for x_, c in xx:
                nc.tensor.matmul(g[0:pv, c:c + 512], sv[0:kp, 0:pv],
                                 x_[0:kp, 0:W], start=True, stop=False)
            for x_, c in xx:
                nc.tensor.matmul(g[0:pv, c:c + 512], svn[0:kp, 0:pv],
                                 x_[0:kp, 2:2 + W], start=False, stop=True)
            for x_, c in xx:
                nc.tensor.matmul(g[0:pv, c + 512:c + 1024], sdf[0:kp, 0:pv],
                                 x_[0:kp, 0:W], start=True, stop=False)
                nc.tensor.matmul(g[0:pv, c + 512:c + 1024], sdf[0:kp, 0:pv],
                                 x_[0:kp, 2:2 + W], start=False, stop=False)
            for x_, c in xx:
                nc.tensor.matmul(g[0:pv, c + 512:c + 1024], sdf2[0:kp, 0:pv],
                                 x_[0:kp, 1:1 + W], start=False, stop=True)

        loop_ctx = tc.For_i(0, loop, 1) if loop > 1 else None
        if loop_ctx is not None:
            loop_ctx.__enter__()

        if variant != "mm":
            emit_dmas()

        # Deferred per-iteration stages, emitted with a lag so engines
        # never wait on each other within an iteration.
        pending = []          # (sq, pv, it) waiting for madd/qmul
        qtiles = {}           # batch index -> q tile
        qfill = {}            # batch index -> number of filled slots

        def do_madd_qmul(sq, pv, it):
            m = m_pool.tile([128, 1024], F16)
            sqv = sq.rearrange("p (a b c) -> p a b c", a=2, b=2, c=512)
            mv = m.rearrange("p (a c) -> p a c", a=2, c=512)
            nc.vector.tensor_add(mv[0:pv, :, :], sqv[0:pv, :, 0, :],
                                 sqv[0:pv, :, 1, :])
            b, slot = divmod(it, QB)
            if slot == 0:
                qtiles[b] = q_pool.tile([128, QB * 512], F16, name=f"q{b}")
            q = qtiles[b]
            qfill[b] = slot + 1
            nc.vector.tensor_mul(q[0:pv, slot * 512:slot * 512 + 512],
                                 m[0:pv, 0:512], m[0:pv, 512:1024])
            if pv < 126:
                # zero unused partitions so the batched sqrt+accum over
                # [0:126] rows stays clean (packed-bottom iters, pv=32);
                # memset APs must start 32-aligned and span <= 32 partitions
                for p0, p1 in ((32, 64), (64, 96), (96, 126)):
                    nc.vector.memset(q[p0:p1, slot * 512:slot * 512 + 512], 0.0)

        def do_sqrt(b):
            q = qtiles.pop(b)
            w = qfill.pop(b) * 512
            nc.scalar.activation(q[0:126, 0:w], q[0:126, 0:w],
                                 mybir.ActivationFunctionType.Sqrt,
                                 accum_out=acc_b[0:126, b:b + 1])

        it = 0
        for phase in range(N_IMG + 2):
            if phase < N_IMG:
                img = phase
                bands = range(N_BANDS)
            else:
                bands = (-1,)
            for t in bands:
                if t >= 0:
                    xlr = blk(img, 0, t)
                    xpr = blk(img, 1, t)
                    stat, pv, kp = (BV, BVN, BDF, BDF2), BAND, 128
                else:
                    q2 = phase - N_IMG
                    xlr = xmt[:, (2 * q2) * BLK:(2 * q2) * BLK + BLK]
                    xpr = xmt[:, (2 * q2 + 1) * BLK:(2 * q2 + 1) * BLK + BLK]
                    stat, pv, kp = (BVM, BVNM, BDFM, BDF2M), 32, 58

                if variant == "dma":
                    it += 1
                    continue
                # PSUM layout: [gx_l | gy_l | gx_p | gy_p], 512 f32 each.
                g = psum_pool.tile([128, 2048], F32)
                emit_mms(g, xlr, xpr, stat, pv, kp)

                if variant in ("dma_mm", "mm"):
                    # cheap PSUM release: copy 64 cols per bank-pair
                    sqm = sq_pool.tile([128, 2048], F16)
                    nc.vector.tensor_copy(
                        sqm[0:126, 0:64], g[0:126, 0:64])
                    nc.vector.tensor_copy(
                        sqm[0:126, 1024:1088], g[0:126, 1024:1088])
                    nc.vector.tensor_reduce(
                        acc_a[0:126, it:it + 1], sqm[0:126, 0:64],
                        axis=mybir.AxisListType.X, op=mybir.AluOpType.add)
                    it += 1
                    continue
                if variant == "evict_actonly":
                    sqa = sq_pool.tile([128, 2048], F16)
                    nc.scalar.activation(sqa[0:pv, :], g[0:pv, :],
                                         mybir.ActivationFunctionType.Square,
                                         accum_out=acc_a[0:pv, it:it + 1])
                    it += 1
                    continue
                if variant == "evict_dveonly":
                    sqd = sq_pool.tile([128, 2048], F16)
                    nc.vector.tensor_copy(sqd[0:pv, :], g[0:pv, :])
                    nc.vector.scalar_tensor_tensor(
                        out=sqd[0:pv, 0:1024], in0=sqd[0:pv, 0:1024],
                        scalar=1.0, in1=sqd[0:pv, 0:1024],
                        op0=mybir.AluOpType.mult, op1=mybir.AluOpType.mult,
                        accum_out=acc_c[0:pv, it:it + 1])
                    it += 1
                    continue
                # Eviction: ACT squares the whole PSUM tile in one batched
                # op (+accum of sum(gx^2+gy^2)). The DVE never touches PSUM:
                # its PSUM reads measurably serialize against PE matmuls in
                # this toolchain, while ACT-PSUM reads overlap them cleanly.
                sq = sq_pool.tile([128, 2048], F16)
                nc.scalar.activation(sq[0:pv, :], g[0:pv, :],
                                     mybir.ActivationFunctionType.Square,
                                     accum_out=acc_a[0:pv, it:it + 1])

                if variant == "evict":
                    it += 1
                    continue
                # Lagged DVE madd/qmul (previous iteration's sq).
                pending.append((sq, pv, it))
                if len(pending) > 1:
                    do_madd_qmul(*pending.pop(0))
                # Lagged ACT sqrt over completed q batches.
                bq = (it - QB - SQRT_LAG) // QB
                if it % QB == SQRT_LAG and bq >= 0 and bq in qtiles:
                    do_sqrt(bq)
                it += 1

        while pending:
            do_madd_qmul(*pending.pop(0))
        for b in sorted(qtiles):
            do_sqrt(b)
        if variant == "dma":
            # consume the tiles so the DMAs stay live
            for j in range(2 * N_IMG):
                nc.vector.tensor_reduce(
                    acc_a[:, j // 2:j // 2 + 1], st[j][:, 0:256],
                    axis=mybir.AxisListType.X, op=mybir.AluOpType.add)
            nc.vector.tensor_reduce(acc_a[:, 8:9], xmt[:, :],
                                    axis=mybir.AxisListType.X,
                                    op=mybir.AluOpType.add)

        if loop_ctx is not None:
            loop_ctx.__exit__(None, None, None)
        nc.vector.tensor_reduce(out_s[:, 0:1], acc_a[:, :],
                                axis=mybir.AxisListType.X, op=mybir.AluOpType.add)
        nc.vector.tensor_reduce(out_s[:, 1:2], acc_b[:, :],
                                axis=mybir.AxisListType.X, op=mybir.AluOpType.add)
        nc.vector.tensor_reduce(out_s[:, 2:3], acc_c[:, :],
                                axis=mybir.AxisListType.X, op=mybir.AluOpType.add)
        nc.sync.dma_start(out=out[:, :], in_=out_s[:, :])
    return _patch_serialization(nc)


_NC = None


def kernel(probs, labels):
    global _NC
    from concourse.bass_utils import run_bass_kernel_spmd

    if _NC is None:
        _NC = build_kernel()

    p = np.asarray(probs)[:, 1:5].astype(np.float16)
    l = np.asarray(labels)[:, 1:5].astype(np.float16)
    wmat = _stationaries()

    in_maps = []
    for k in range(8):
        l4 = l[2 * k:2 * k + 2].reshape(N_IMG, H, W)
        p4 = p[2 * k:2 * k + 2].reshape(N_IMG, H, W)
        xs_h, xm_h = pack_host_inputs(l4, p4)
        in_maps.append({"xs": xs_h, "xmh": xm_h, "consts": wmat})
    res = run_bass_kernel_spmd(_NC, in_maps, list(range(8)))
    total = 0.0
    for r in res.results:
        o = r["out"].astype(np.float64)
        total += o[:, 0].sum() + o[:, 2].sum() - 2.0 * o[:, 1].sum()
    return np.float32(total / (16 * H * W))
